# revision 4
# baseline (speedup 1.0000x reference)
"""Antialiased 2x upsampling (StyleGAN2 upsample_2d, k=[1,3,3,1], factor=2).

Input  x: (8, 256, 256, 64) f32 NHWC  ->  output: (8, 511, 511, 64) f32.

Math (separable, polyphase). Host pre-scales x by 1/16 and casts to bf16,
so with A[i] = x'[i-1] (zero-padded), B[i] = x'[i]  (x' = x/16, bf16):
  g3 = A + 3B     (= (1/16)x[i-1] + (3/16)x[i])
  h3 = 3A + B
  g9 = 3*g3, h9 = 3*h3
  out[2i,   2j]   = g9[j]   + g3[j-1]
  out[2i,   2j-1] = g9[j-1] + g3[j]
  out[2i-1, 2j]   = h9[j]   + h3[j-1]
  out[2i-1, 2j-1] = h9[j-1] + h3[j]

Sharding: pure data parallel, one batch image per NeuronCore (8 cores).
Layout: partition dim = input row i, free dim = w*C+c. All shifts are
free-dim AP offsets except the H-shift, realized by loading a row-shifted
second copy (A) of each input tile from DRAM.

Performance notes (measured on TRN2):
- DRAM x and out are bf16: the host casts f32->bf16 (and pre-scales by
  1/16); the kernel computed in bf16 anyway, so this halves HBM traffic
  at no extra error. rel err ~3e-3 (gate is 2e-2).
- No in-flight dtype cast -> all DMAs are HWDGE. They all go on ONE
  queue (nc.sync): with loads and stores on separate queues each SDMA
  engine round-robins between queues at packet granularity and per-packet
  switch bubbles cost ~30% of HBM bandwidth.
- Row 0 is folded into the first h-tile: DMA descriptors can target
  arbitrary SBUF partition starts (only compute APs are restricted to
  0/32/64/96), so A[1:128] <- x[0:127] with A[0] memset to zero, and the
  first odd-row store reads rb from partition 1. A separate 1-partition
  row-0 pass would cost full-FD time on every engine it touches.
- The 1/16 host prescale folds the blur-tap scales into single
  scalar_tensor_tensor ops (scalar=3); ACT runs the two scale-by-3
  copies, and one of the two stt ops runs on GPSIMD so DVE only carries
  one stt plus the W-pass adds.
- W-pass = tensor_tensor adds of two pre-scaled copies: plain adds
  hit the DVE 2x bf16 packing mode; scalar_tensor_tensor does not.
- Loads are issued PRE iterations ahead of compute so store-emission
  waits on the single DMA queue never starve the load stream.
"""

import numpy as np
import ml_dtypes

import concourse.bacc as bacc
import concourse.mybir as mybir
from concourse.tile import TileContext
from concourse.bass_utils import run_bass_kernel_spmd

F32 = mybir.dt.float32
BF16 = mybir.dt.bfloat16
MULT = mybir.AluOpType.mult
ADD = mybir.AluOpType.add

B_FULL, H_FULL, W_FULL, C_FULL = 8, 256, 256, 64
N_CORES = 8


def build_upsample_tile(tc, out, x, H, W, C, P, WT, SBDT=BF16):
    """Trace the upsampling kernel into TileContext tc.

    x:   DRAM AP [H, W*C]   (bf16, pre-scaled by 1/16 on host)
    out: DRAM AP [2H-1, (2W-1)*C]  (bf16)
    P:   partition tile height (input rows per tile)
    WT:  input cols per w-tile
    """
    nc = tc.nc
    assert W % WT == 0 and H % P == 0
    n_wt = W // WT
    FW = (WT + 1) * C  # tile free width: cols w0-1 .. w0+WT-1

    # h-tiles cover input rows i = i0 .. i0+P-1 (partition p <-> i = i0+p).
    # Row i produces out rows 2i-1 (odd, absent for i=0) and 2i (even).
    h_tiles = [(i0, P) for i0 in range(0, H, P)]

    seg = 2 * WT * C  # one output row segment (2*WT cols)

    with (
        tc.tile_pool(name="io", bufs=2) as io_pool,
        tc.tile_pool(name="mid", bufs=1) as mid_pool,
        tc.tile_pool(name="rb", bufs=2) as rb_pool,
    ):
        def v(t, qlo, PT):
            return t[:PT, qlo * C : (qlo + WT) * C].rearrange("p (j c) -> p j c", c=C)

        def wpass(f9, f3, rbv, s, PT):
            # out[r, 2j]   = f9[j]   + f3[j-1]   (even cols -> q=1 slot)
            # plain tensor_tensor adds of pre-scaled copies: eligible for the
            # DVE 2x bf16 packing mode (scalar_tensor_tensor is not)
            nc.vector.tensor_add(
                out=rbv[:PT, s, :, 1, :], in0=v(f9, 1, PT), in1=v(f3, 0, PT)
            )
            # out[r, 2j-1] = f9[j-1] + f3[j]     (odd cols -> q=0 slot)
            nc.vector.tensor_add(
                out=rbv[:PT, s, :, 0, :], in0=v(f9, 0, PT), in1=v(f3, 1, PT)
            )

        def wparams(wt):
            w0 = wt * WT
            return dict(
                w0=w0,
                cl=(w0 - 1) * C,
                skip=C if w0 == 0 else 0,
                dcol_lo=0 if w0 == 0 else (2 * w0 - 1) * C,
                dw=seg - (C if w0 == 0 else 0),
                ld_w=WT * C if w0 == 0 else FW,
                ld_off=C if w0 == 0 else 0,
            )

        def pchunks(PT, q_lo=0):
            # 64-partition chunks measured fastest; q_lo=1 for the first
            # tile's shifted loads / odd stores (no out row -1)
            return [(q0, q1) for q0, q1 in ((q_lo, 64), (64, PT)) if q1 > q0]

        # --- main tiles, software-pipelined: loads issued PRE iterations
        # ahead of compute so the single DMA queue's wait-for-compute
        # (before each store emission) never blocks the next loads.
        steps = [(ti, wt) for wt in range(n_wt) for ti in range(len(h_tiles))]
        N = len(steps)
        PRE = 2
        tiles = {}

        def load(s):
            ti, wt = steps[s]
            i0, PT = h_tiles[ti]
            p = wparams(wt)
            lo, lw = p["ld_off"], p["ld_w"]
            # A[q] = x[i0+q-1], B[q] = x[i0+q]; split into 64-partition DMAs
            # so concurrent one-packet transfers spread across SDMA engines.
            A = io_pool.tile([PT, FW], SBDT, tag="A", name=f"A_{ti}_{wt}")
            Bt = io_pool.tile([PT, FW], SBDT, tag="B", name=f"B_{ti}_{wt}")
            if p["w0"] == 0:
                nc.vector.memset(A[:PT, 0:C], 0.0)
                nc.vector.memset(Bt[:PT, 0:C], 0.0)
            if ti == 0:
                # x[-1] = 0: zero partition 0 and load the shifted copy into
                # partitions 1..P-1 (DMA may start at any partition).
                nc.vector.memset(A[0:1, :], 0.0)
            for q0, q1 in pchunks(PT, q_lo=1 if ti == 0 else 0):
                nc.sync.dma_start(
                    out=A[q0:q1, lo : lo + lw],
                    in_=x[i0 - 1 + q0 : i0 - 1 + q1,
                          p["cl"] + lo : p["cl"] + lo + lw],
                )
            for q0, q1 in pchunks(PT):
                nc.sync.dma_start(
                    out=Bt[q0:q1, lo : lo + lw],
                    in_=x[i0 + q0 : i0 + q1, p["cl"] + lo : p["cl"] + lo + lw],
                )
            tiles[s] = (A, Bt)

        def compute_store(s):
            ti, wt = steps[s]
            i0, PT = h_tiles[ti]
            p = wparams(wt)
            A, Bt = tiles.pop(s)
            A = A[:PT, :]
            Bt = Bt[:PT, :]

            # g3 = A + 3B, h3 = 3A + B  (input pre-scaled by 1/16).
            # The x3 pre-scales run on DVE tensor_scalar (4x bf16 mode, so
            # nearly free); the adds split DVE/GPSIMD (GPSIMD has no stt or
            # tensor_scalar in the Pool ISA, but tensor_tensor works) so DVE
            # keeps headroom for the 2x-mode W-adds.
            t3a = mid_pool.tile([PT, FW], SBDT, tag="t3a", name=f"t3a_{ti}_{wt}")
            t3b = mid_pool.tile([PT, FW], SBDT, tag="t3b", name=f"t3b_{ti}_{wt}")
            nc.vector.tensor_scalar_mul(t3a[:], A, 3.0)
            nc.vector.tensor_scalar_mul(t3b[:], Bt, 3.0)
            g3 = mid_pool.tile([PT, FW], SBDT, tag="g3", name=f"g3_{ti}_{wt}")
            h3 = mid_pool.tile([PT, FW], SBDT, tag="h3", name=f"h3_{ti}_{wt}")
            nc.vector.tensor_add(out=g3[:], in0=t3b[:], in1=A)
            nc.gpsimd.tensor_add(out=h3[:], in0=t3a[:], in1=Bt)
            g9 = mid_pool.tile([PT, FW], SBDT, tag="g9", name=f"g9_{ti}_{wt}")
            h9 = mid_pool.tile([PT, FW], SBDT, tag="h9", name=f"h9_{ti}_{wt}")
            nc.scalar.mul(g9[:], g3[:], 3.0)
            nc.scalar.mul(h9[:], h3[:], 3.0)

            # rowbuf: [odd-row seg | even-row seg] so DRAM rows ascend;
            # each seg = WT x [oddcol | evencol] x C
            rb = rb_pool.tile([PT, 4 * WT * C], SBDT, tag="rb", name=f"rb_{ti}_{wt}")
            rbv = rb.rearrange("p (s j q c) -> p s j q c", s=2, j=WT, q=2, c=C)
            wpass(h9, h3, rbv, 0, PT)  # odd rows 2i-1 -> first segment
            wpass(g9, g3, rbv, 1, PT)  # even rows 2i -> second segment

            # stores: odd rows 2(i0+q)-1 and even rows 2(i0+q), split into
            # 64-partition one-packet DMAs like the loads. For the first
            # h-tile the odd store starts at partition 1 (no out row -1).
            for q0, q1 in pchunks(PT, q_lo=1 if ti == 0 else 0):
                r0 = 2 * (i0 + q0) - 1
                nc.sync.dma_start(
                    out=out[r0 : r0 + 2 * (q1 - q0) - 1 : 2,
                            p["dcol_lo"] : p["dcol_lo"] + p["dw"]],
                    in_=rb[q0:q1, p["skip"] : seg],
                )
            for q0, q1 in pchunks(PT):
                r0 = 2 * (i0 + q0)
                nc.sync.dma_start(
                    out=out[r0 : r0 + 2 * (q1 - q0) - 1 : 2,
                            p["dcol_lo"] : p["dcol_lo"] + p["dw"]],
                    in_=rb[q0:q1, seg + p["skip"] : 2 * seg],
                )

        for s in range(N + PRE):
            if s < N:
                load(s)
            if s >= PRE:
                compute_store(s - PRE)


def build_nc(H=H_FULL, W=W_FULL, C=C_FULL, P=128, WT=64):
    nc = bacc.Bacc("TRN2", target_bir_lowering=False, debug=False)
    x = nc.declare_dram_parameter("x", [H, W * C], BF16, isOutput=False).ap()
    out = nc.declare_dram_parameter(
        "out", [2 * H - 1, (2 * W - 1) * C], BF16, isOutput=True
    ).ap()
    with TileContext(nc) as tc:
        build_upsample_tile(tc, out, x, H, W, C, P, WT, SBDT=BF16)
    nc.compile()
    return nc


_NC_CACHE = {}


def _get_nc():
    key = (H_FULL, W_FULL, C_FULL)
    if key not in _NC_CACHE:
        _NC_CACHE[key] = build_nc()
    return _NC_CACHE[key]


def run_spmd(x, trace=False, **kwargs):
    """x: (8, 256, 256, 64) f32. Returns (BassKernelResults, out (8,511,511,64))."""
    nc = _get_nc()
    # Pre-scale by 1/16 (exact) and cast to bf16 on the host: the kernel's
    # blur taps become {1, 3, 9} so every scale is a single exact op.
    xs = (np.asarray(x, dtype=np.float32) * (1.0 / 16.0)).astype(ml_dtypes.bfloat16)
    in_maps = [
        {"x": np.ascontiguousarray(xs[b]).reshape(H_FULL, W_FULL * C_FULL)}
        for b in range(N_CORES)
    ]
    res = run_bass_kernel_spmd(
        nc, in_maps, core_ids=list(range(N_CORES)), trace=trace, **kwargs
    )
    out = np.stack(
        [
            res.results[b]["out"]
            .astype(np.float32)
            .reshape(2 * H_FULL - 1, 2 * W_FULL - 1, C_FULL)
            for b in range(N_CORES)
        ]
    )
    return res, out


def kernel(x):
    x = np.asarray(x, dtype=np.float32)
    _, out = run_spmd(x, trace=False)
    return out


# revision 5
# speedup vs baseline: 1.1041x; 1.1041x over previous
"""Antialiased 2x upsampling (StyleGAN2 upsample_2d, k=[1,3,3,1], factor=2).

Input  x: (8, 256, 256, 64) f32 NHWC  ->  output: (8, 511, 511, 64) f32.

Math (separable, polyphase). Host pre-scales x by 1/16, casts to bf16 and
prepends a zero row, so with A[i] = xp[i] (= x'[i-1]), B[i] = xp[i+1]
(= x'[i], x' = x/16):
  g3 = A + 3B     (= (1/16)x[i-1] + (3/16)x[i])
  h3 = 3A + B
  g9 = 3*g3, h9 = 3*h3
  out[2i,   2j]   = g9[j]   + g3[j-1]
  out[2i,   2j-1] = g9[j-1] + g3[j]
  out[2i-1, 2j]   = h9[j]   + h3[j-1]
  out[2i-1, 2j-1] = h9[j-1] + h3[j]

Sharding: pure data parallel, one batch image per NeuronCore (8 cores).
Layout: partition dim = input row i, free dim = w*C+c. All shifts are
free-dim AP offsets except the H-shift, realized by loading a row-shifted
second copy (A) of each input tile from DRAM (the zero pad row makes this
uniform for the first tile).

Performance notes (measured on TRN2):
- DRAM x and out are bf16: the host casts f32->bf16 (and pre-scales by
  1/16); the kernel computed in bf16 anyway, so this halves HBM traffic
  at no extra error. rel err ~4e-3 (gate is 2e-2).
- All bulk DMAs go through gpsimd (SWDGE): HWDGE (sync/scalar) DMAs
  measured only ~17 GB/s per SDMA engine vs ~24 GB/s for SWDGE on this
  access pattern, regardless of queue splitting. GPSIMD therefore does
  no compute (Q7 cores emit descriptors), and its SBUF traffic would
  also degrade concurrent DVE 2x-mode ops.
- Out row 0 (i=0 has no odd output row) is handled by starting the
  first tile's odd-row store at SBUF partition 1: DMA descriptors may
  start at any partition (only compute APs are restricted to 0/32/64/96).
- The 1/16 host prescale folds the blur-tap scales into single
  scalar_tensor_tensor ops (scalar=3), so ACT only runs 2 scale-by-3
  copies per tile instead of 4 muls.
- W-pass = tensor_tensor adds of two pre-scaled copies: plain adds
  hit the DVE 2x bf16 packing mode; scalar_tensor_tensor does not.
- Loads are issued PRE iterations ahead of compute so the store-emission
  waits on the single SWDGE queue never starve the load stream.
"""

import numpy as np
import ml_dtypes

import concourse.bacc as bacc
import concourse.mybir as mybir
from concourse.tile import TileContext
from concourse.bass_utils import run_bass_kernel_spmd

F32 = mybir.dt.float32
BF16 = mybir.dt.bfloat16
MULT = mybir.AluOpType.mult
ADD = mybir.AluOpType.add

B_FULL, H_FULL, W_FULL, C_FULL = 8, 256, 256, 64
N_CORES = 8


def build_upsample_tile(tc, out, x, H, W, C, P, WT, SBDT=BF16):
    """Trace the upsampling kernel into TileContext tc.

    x:   DRAM AP [H+1, W*C]  (bf16, pre-scaled by 1/16, row 0 = zeros)
    out: DRAM AP [2H-1, (2W-1)*C]  (bf16)
    P:   partition tile height (input rows per tile)
    WT:  input cols per w-tile
    """
    nc = tc.nc
    assert W % WT == 0 and H % P == 0
    n_wt = W // WT
    FW = (WT + 1) * C  # tile free width: cols w0-1 .. w0+WT-1

    # h-tiles cover input rows i = i0 .. i0+P-1 (partition p <-> i = i0+p).
    # Row i produces out rows 2i-1 (odd, absent for i=0) and 2i (even).
    h_tiles = [(i0, P) for i0 in range(0, H, P)]

    seg = 2 * WT * C  # one output row segment (2*WT cols)

    with (
        tc.tile_pool(name="io", bufs=2) as io_pool,
        tc.tile_pool(name="mid", bufs=1) as mid_pool,
        tc.tile_pool(name="rb", bufs=2) as rb_pool,
    ):
        def v(t, qlo, PT):
            return t[:PT, qlo * C : (qlo + WT) * C].rearrange("p (j c) -> p j c", c=C)

        def wpass(f9, f3, rbv, s, PT):
            # out[r, 2j]   = f9[j]   + f3[j-1]   (even cols -> q=1 slot)
            # plain tensor_tensor adds of pre-scaled copies: eligible for the
            # DVE 2x bf16 packing mode (scalar_tensor_tensor is not)
            nc.vector.tensor_add(
                out=rbv[:PT, s, :, 1, :], in0=v(f9, 1, PT), in1=v(f3, 0, PT)
            )
            # out[r, 2j-1] = f9[j-1] + f3[j]     (odd cols -> q=0 slot)
            nc.vector.tensor_add(
                out=rbv[:PT, s, :, 0, :], in0=v(f9, 0, PT), in1=v(f3, 1, PT)
            )

        def wparams(wt):
            w0 = wt * WT
            return dict(
                w0=w0,
                cl=(w0 - 1) * C,
                skip=C if w0 == 0 else 0,
                dcol_lo=0 if w0 == 0 else (2 * w0 - 1) * C,
                dw=seg - (C if w0 == 0 else 0),
                ld_w=WT * C if w0 == 0 else FW,
                ld_off=C if w0 == 0 else 0,
            )

        def pchunks(PT, q_lo=0):
            # legal SBUF partition starts for compute are 0/32/64/96;
            # 64-partition DMA chunks measured fastest. q_lo=1 for the
            # first tile's odd-row store (no out row -1).
            return [(q0, q1) for q0, q1 in ((q_lo, 64), (64, PT)) if q1 > q0]

        # --- main tiles, software-pipelined: loads issued PRE iterations
        # ahead of compute so the gpsimd queue's wait-for-compute (before
        # each store emission) never blocks the next loads.
        steps = [(ti, wt) for wt in range(n_wt) for ti in range(len(h_tiles))]
        N = len(steps)
        PRE = 2
        tiles = {}

        def load(s):
            ti, wt = steps[s]
            i0, PT = h_tiles[ti]
            p = wparams(wt)
            lo, lw = p["ld_off"], p["ld_w"]
            # A[q] = xp[i0+q], B[q] = xp[i0+q+1]; split into 64-partition
            # DMAs so concurrent one-packet transfers spread across engines.
            A = io_pool.tile([PT, FW], SBDT, tag="A", name=f"A_{ti}_{wt}")
            Bt = io_pool.tile([PT, FW], SBDT, tag="B", name=f"B_{ti}_{wt}")
            if p["w0"] == 0:
                nc.vector.memset(A[:PT, 0:C], 0.0)
                nc.vector.memset(Bt[:PT, 0:C], 0.0)
            for q0, q1 in pchunks(PT):
                nc.gpsimd.dma_start(
                    out=A[q0:q1, lo : lo + lw],
                    in_=x[i0 + q0 : i0 + q1, p["cl"] + lo : p["cl"] + lo + lw],
                )
            for q0, q1 in pchunks(PT):
                nc.gpsimd.dma_start(
                    out=Bt[q0:q1, lo : lo + lw],
                    in_=x[i0 + 1 + q0 : i0 + 1 + q1,
                          p["cl"] + lo : p["cl"] + lo + lw],
                )
            tiles[s] = (A, Bt)

        def compute_store(s):
            ti, wt = steps[s]
            i0, PT = h_tiles[ti]
            p = wparams(wt)
            A, Bt = tiles.pop(s)
            A = A[:PT, :]
            Bt = Bt[:PT, :]

            # g3 = A + 3B, h3 = 3A + B  (input pre-scaled by 1/16)
            g3 = mid_pool.tile([PT, FW], SBDT, tag="g3", name=f"g3_{ti}_{wt}")
            h3 = mid_pool.tile([PT, FW], SBDT, tag="h3", name=f"h3_{ti}_{wt}")
            nc.vector.scalar_tensor_tensor(
                out=g3[:], in0=Bt, scalar=3.0, in1=A, op0=MULT, op1=ADD
            )
            nc.vector.scalar_tensor_tensor(
                out=h3[:], in0=A, scalar=3.0, in1=Bt, op0=MULT, op1=ADD
            )
            g9 = mid_pool.tile([PT, FW], SBDT, tag="g9", name=f"g9_{ti}_{wt}")
            h9 = mid_pool.tile([PT, FW], SBDT, tag="h9", name=f"h9_{ti}_{wt}")
            nc.scalar.mul(g9[:], g3[:], 3.0)
            nc.scalar.mul(h9[:], h3[:], 3.0)

            # rowbuf: [odd-row seg | even-row seg] so DRAM rows ascend;
            # each seg = WT x [oddcol | evencol] x C
            rb = rb_pool.tile([PT, 4 * WT * C], SBDT, tag="rb", name=f"rb_{ti}_{wt}")
            rbv = rb.rearrange("p (s j q c) -> p s j q c", s=2, j=WT, q=2, c=C)
            wpass(h9, h3, rbv, 0, PT)  # odd rows 2i-1 -> first segment
            wpass(g9, g3, rbv, 1, PT)  # even rows 2i -> second segment

            # stores: odd rows 2(i0+q)-1 and even rows 2(i0+q), split into
            # 64-partition one-packet DMAs like the loads. For the first
            # h-tile the odd store starts at partition 1 (no out row -1).
            for q0, q1 in pchunks(PT, q_lo=1 if ti == 0 else 0):
                r0 = 2 * (i0 + q0) - 1
                nc.gpsimd.dma_start(
                    out=out[r0 : r0 + 2 * (q1 - q0) - 1 : 2,
                            p["dcol_lo"] : p["dcol_lo"] + p["dw"]],
                    in_=rb[q0:q1, p["skip"] : seg],
                )
            for q0, q1 in pchunks(PT):
                r0 = 2 * (i0 + q0)
                nc.gpsimd.dma_start(
                    out=out[r0 : r0 + 2 * (q1 - q0) - 1 : 2,
                            p["dcol_lo"] : p["dcol_lo"] + p["dw"]],
                    in_=rb[q0:q1, seg + p["skip"] : 2 * seg],
                )

        for s in range(N + PRE):
            if s < N:
                load(s)
            if s >= PRE:
                compute_store(s - PRE)


def build_nc(H=H_FULL, W=W_FULL, C=C_FULL, P=128, WT=64):
    nc = bacc.Bacc(
        "TRN2", target_bir_lowering=False, debug=False,
        dynamic_dma_scratch_size=16384,
    )
    x = nc.declare_dram_parameter("x", [H + 1, W * C], BF16, isOutput=False).ap()
    out = nc.declare_dram_parameter(
        "out", [2 * H - 1, (2 * W - 1) * C], BF16, isOutput=True
    ).ap()
    with TileContext(nc) as tc:
        build_upsample_tile(tc, out, x, H, W, C, P, WT, SBDT=BF16)
    nc.compile()
    return nc


_NC_CACHE = {}


def _get_nc():
    key = (H_FULL, W_FULL, C_FULL)
    if key not in _NC_CACHE:
        _NC_CACHE[key] = build_nc()
    return _NC_CACHE[key]


def run_spmd(x, trace=False, **kwargs):
    """x: (8, 256, 256, 64) f32. Returns (BassKernelResults, out (8,511,511,64))."""
    nc = _get_nc()
    # Pre-scale by 1/16 (exact) and cast to bf16 on the host: the kernel's
    # blur taps become {1, 3, 9} so every scale is a single exact op.
    # Row 0 of the padded input is the x[-1] = 0 boundary row.
    xs = (np.asarray(x, dtype=np.float32) * (1.0 / 16.0)).astype(ml_dtypes.bfloat16)
    xp = np.zeros((N_CORES, H_FULL + 1, W_FULL * C_FULL), dtype=ml_dtypes.bfloat16)
    xp[:, 1:, :] = xs.reshape(N_CORES, H_FULL, W_FULL * C_FULL)
    in_maps = [{"x": np.ascontiguousarray(xp[b])} for b in range(N_CORES)]
    res = run_bass_kernel_spmd(
        nc, in_maps, core_ids=list(range(N_CORES)), trace=trace, **kwargs
    )
    out = np.stack(
        [
            res.results[b]["out"]
            .astype(np.float32)
            .reshape(2 * H_FULL - 1, 2 * W_FULL - 1, C_FULL)
            for b in range(N_CORES)
        ]
    )
    return res, out


def kernel(x):
    x = np.asarray(x, dtype=np.float32)
    _, out = run_spmd(x, trace=False)
    return out
